# revision 1
# baseline (speedup 1.0000x reference)
"""ContextMatching kernel for Trainium2, 8-core SPMD.

Math: the reference computes softmax over j of s[b,i,j] = p1[b,i] + p2[b,j]
(masked to i < l1[b], j < l2[b]) and contracts the attention weights with
s2.  Because the score is additive, the row term p1[b,i] cancels inside the
softmax, so every valid row i shares the same attention vector

    alpha[b,j] = exp(p2[b,j]) / sum_{j' < l2[b]} exp(p2[b,j'])

(no max subtraction needed: |p2| <= ||s2_row||*||w2|| ~ 13, exp is safe in
f32) and the output collapses to

    out[b,i,:] = (i < l1[b]) ? sum_{j < l2[b]} alpha[b,j] * s2[b,j,:] : 0.

s1 never influences the output.

Implementation notes:
  * Data-parallel over batch, B=16 -> 2 batches per core, paired to balance
    the number of valid 128-row j-chunks (NT tasks per core, uniform across
    cores so one SPMD program serves all 8 cores).
  * The host packs only the valid j-chunks of s2 into a [128, NT*C] bf16
    tensor and precomputes the batch-membership/validity masks and the
    output row index table, so the device does no iota/compare work.
  * Everything on device is bf16 except the f32 accumulators (harness
    tolerance is 2e-2; measured error ~3e-3).  bf16 gives 2x DVE, 4x PE
    and half the DMA bytes vs f32.
  * Per chunk-task t: p2[:,t] = <s2_t, w2> via one fused
    scalar_tensor_tensor (out=(in0*1)*in1, accum_out=row sum), alternating
    DVE / GpSimd so two engines chew the dot products in parallel; exp on
    ACT; two tiny DVE muls build the masked e columns for both batches;
    one PE matmul per 512-col half accumulates ctx[2, C] in PSUM.
  * d_b = sum(e) via column reduce + rank-1 matmul; 1/d folded into the
    PSUM->SBUF copy of ctx (ACT scale); ctx rows broadcast to 128
    partitions with selector-matrix PE matmuls.
  * Output: rows i < l1 all equal cbs_b, rows >= l1 are zero.  The PJRT
    path donates zero-initialized output buffers (documented contract,
    both native and axon paths), so the kernel writes ONLY the valid rows:
    one indirect (scatter) DMA per batch scatters the cbs_b rows to
    host-computed row indices; invalid rows carry an out-of-bounds index
    and are silently skipped (oob_is_err=False).
"""

import numpy as np
import ml_dtypes

BF16 = ml_dtypes.bfloat16

B, T1, T2, C = 16, 1024, 1024, 1024
N_CORES = 8
BPC = B // N_CORES  # batches per core
P = 128
NRO = T1 // P  # output row chunks per batch
OOB = 1 << 20  # sentinel row index: skipped by bounds check

_cached = {}  # NT -> program
last_results = None  # BassKernelResults of the most recent run (for profiling)


def _build_program(NT):
    import concourse.mybir as mybir
    import concourse.tile as tile
    from concourse import bacc, bass

    f32 = mybir.dt.float32
    bf16 = mybir.dt.bfloat16
    i32 = mybir.dt.int32
    Alu = mybir.AluOpType
    Act = mybir.ActivationFunctionType
    Axis = mybir.AxisListType

    nc = bacc.Bacc(None, target_bir_lowering=False, name="context_matching")

    s2cd = nc.dram_tensor("s2c", [P, NT * C], bf16, kind="ExternalInput")
    w2bd = nc.dram_tensor("w2b", [P, C], bf16, kind="ExternalInput")
    auxd = nc.dram_tensor("aux", [P, 2 * NT], bf16, kind="ExternalInput")
    seld = nc.dram_tensor("sel", [2, BPC * P], bf16, kind="ExternalInput")
    idxd = nc.dram_tensor("idx", [P, BPC * NRO], i32, kind="ExternalInput")
    outd = nc.dram_tensor("out", [BPC * T1, C], bf16, kind="ExternalOutput")

    # task groups of two (pipeline granularity)
    groups = [list(range(g, min(g + 2, NT))) for g in range(0, NT, 2)]

    with tile.TileContext(nc) as tc:
        with (
            tc.tile_pool(name="statics", bufs=1) as statics,
            tc.tile_pool(name="s2pool", bufs=len(groups)) as s2pool,
            tc.tile_pool(name="scratch", bufs=4) as scratch,
            tc.tile_pool(name="smalls", bufs=1) as smalls,
            tc.tile_pool(name="pctx", bufs=1, space="PSUM") as pctx,
            tc.tile_pool(name="pd2", bufs=1, space="PSUM") as pd2,
            tc.tile_pool(name="pcb", bufs=2, space="PSUM") as pcb,
        ):
            # ---- the big loads first: s2 task groups, then w2 ----
            s2ts = []
            for gi, grp in enumerate(groups):
                gl = len(grp)
                s2t = s2pool.tile([P, gl * C], bf16, name=f"s2t_{gi}", tag="s2t")
                nc.sync.dma_start(
                    out=s2t, in_=s2cd[:, grp[0] * C : (grp[-1] + 1) * C]
                )
                s2ts.append(s2t)
            w2b = statics.tile([P, C], bf16)
            nc.sync.dma_start(out=w2b, in_=w2bd[:, :])

            # ---- small statics ----
            aux = statics.tile([P, 2 * NT], bf16)
            nc.sync.dma_start(out=aux, in_=auxd[:, :])
            selt = statics.tile([2, BPC * P], bf16)
            nc.sync.dma_start(out=selt, in_=seld[:, :])
            sel = [selt[:, b * P : (b + 1) * P] for b in range(BPC)]
            idxt = statics.tile([P, BPC * NRO], i32)
            nc.sync.dma_start(out=idxt, in_=idxd[:, :])

            # ---- persistent smalls ----
            p2f = smalls.tile([P, NT], f32, name="p2f")
            e = smalls.tile([P, NT], bf16, name="e")
            E2 = smalls.tile([P, NT, 2], bf16, name="E2")  # masked e, interleaved
            EBR = smalls.tile([P, 2], f32, name="EBR")
            rinv2 = smalls.tile([2, 1], f32, name="rinv2")
            ctxs = smalls.tile([2, C], bf16, name="ctxs")
            cbs = [smalls.tile([P, C], bf16, name=f"cbs_{b}") for b in range(BPC)]
            ones_c = smalls.tile([P, 1], f32, name="ones_c")
            nc.gpsimd.memset(ones_c, 1.0)

            ctxp = pctx.tile([2, C], f32, name="ctxp")

            # ---- per-group pipeline: p2 -> e -> masked e -> ctx matmul ----
            for gi, grp in enumerate(groups):
                s2t = s2ts[gi]
                for k, t in enumerate(grp):
                    scr = scratch.tile([P, C], bf16, name=f"scr_{t}", tag="scr")
                    nc.vector.scalar_tensor_tensor(
                        out=scr,
                        in0=s2t[:, k * C : (k + 1) * C],
                        scalar=1.0,
                        in1=w2b,
                        op0=Alu.mult,
                        op1=Alu.mult,
                        accum_out=p2f[:, t : t + 1],
                    )
                g0, g1 = grp[0], grp[-1] + 1
                nc.scalar.activation(
                    out=e[:, g0:g1], in_=p2f[:, g0:g1], func=Act.Exp
                )
                for b in range(BPC):
                    nc.gpsimd.tensor_mul(
                        E2[:, g0:g1, b], e[:, g0:g1], aux[:, b * NT + g0 : b * NT + g1]
                    )
                for k, t in enumerate(grp):
                    for h in range(2):
                        cols = slice(h * 512, (h + 1) * 512)
                        nc.tensor.matmul(
                            ctxp[:, cols],
                            lhsT=E2[:, t, :],
                            rhs=s2t[:, k * C + h * 512 : k * C + (h + 1) * 512],
                            start=(t == 0),
                            stop=(t == NT - 1),
                        )

            # ---- d_b = sum_j e (masked), rinv = 1/d ----
            for b in range(BPC):
                nc.vector.reduce_sum(
                    out=EBR[:, b : b + 1], in_=E2[:, :, b], axis=Axis.X
                )
            d2p = pd2.tile([2, 1], f32, name="d2p")
            nc.tensor.matmul(d2p, lhsT=EBR, rhs=ones_c, start=True, stop=True)
            nc.vector.reciprocal(rinv2, d2p)

            # ---- ctxs = (1/d) * ctx  (normalize + f32->bf16 in one ACT op) ----
            nc.scalar.activation(
                out=ctxs, in_=ctxp, func=Act.Copy, scale=rinv2[:, 0:1]
            )

            # ---- broadcast ctxs rows to all 128 partitions (selector PE) ----
            for b in range(BPC):
                cbp = pcb.tile([P, C], f32, name=f"cbp_{b}", tag="cbp")
                for h in range(2):
                    cols = slice(h * 512, (h + 1) * 512)
                    nc.tensor.matmul(
                        cbp[:, cols],
                        lhsT=sel[b],
                        rhs=ctxs[:, cols],
                        start=True,
                        stop=True,
                    )
                nc.scalar.activation(out=cbs[b], in_=cbp, func=Act.Copy)

            # ---- scatter the valid output rows (invalid indices are OOB) ----
            import os
            CW = int(os.environ.get("K_SCW", "1"))  # indices per scatter
            for b in (range(BPC) if "K_NOSCATTER" not in os.environ else []):
                for w0 in range(0, NRO, CW):
                    cw = min(CW, NRO - w0)
                    if cw == 1:
                        src = cbs[b][:, :]
                    else:
                        src = cbs[b][:, :].unsqueeze(1).broadcast_to([P, cw, C])
                    c0 = b * NRO + w0
                    nc.gpsimd.indirect_dma_start(
                        out=outd[:, :],
                        out_offset=bass.IndirectOffsetOnAxis(
                            ap=idxt[:, c0 : c0 + cw], axis=0
                        ),
                        in_=src,
                        in_offset=None,
                        bounds_check=BPC * T1 - 1,
                        oob_is_err=False,
                    )

    nc.finalize()
    return nc


def _plan(l1, l2):
    """Pair batches across cores to balance valid-chunk counts."""
    kj = -(-l2 // P)  # ceil(l2/128), >= 1
    order = np.argsort(-kj, kind="stable")
    pairs = [(int(order[i]), int(order[2 * N_CORES - 1 - i])) for i in range(N_CORES)]
    NT = int(max(kj[a] + kj[b] for a, b in pairs))
    return pairs, kj, NT


def kernel(s1, l1, s2, l2, w):
    global last_results
    from concourse.bass_utils import run_bass_kernel_spmd

    s2 = np.asarray(s2)
    w = np.asarray(w, dtype=np.float32)
    l1 = np.asarray(l1).astype(np.int64).ravel()
    l2 = np.asarray(l2).astype(np.int64).ravel()
    assert s2.shape == (B, T2, C) and w.shape == (1, 2 * C)

    pairs, kj, NT = _plan(l1, l2)
    if NT not in _cached:
        _cached[NT] = _build_program(NT)
    nc = _cached[NT]

    w2b = np.broadcast_to(w[0, C:].astype(BF16), (P, C))
    selh = np.zeros((2, BPC * P), dtype=BF16)
    for b in range(BPC):
        selh[b, b * P : (b + 1) * P] = 1.0
    iot = np.arange(P)

    in_maps = []
    for c in range(N_CORES):
        s2c = np.zeros((P, NT * C), dtype=BF16)
        aux = np.zeros((P, 2 * NT), dtype=BF16)
        idx = np.full((P, BPC * NRO), OOB, dtype=np.int32)
        base_t = 0
        for lb, g in enumerate(pairs[c]):
            for k in range(int(kj[g])):
                t = base_t + k
                j0 = k * P
                s2c[:, t * C : (t + 1) * C] = s2[g, j0 : j0 + P, :]
                aux[:, lb * NT + t] = (j0 + iot) < l2[g]
            base_t += int(kj[g])
            for ww in range(NRO):
                rows = ww * P + iot
                valid = rows < l1[g]
                col = lb * NRO + ww
                idx[valid, col] = lb * T1 + rows[valid]
        in_maps.append({"s2c": s2c, "w2b": w2b, "aux": aux, "sel": selh, "idx": idx})

    last_results = run_bass_kernel_spmd(nc, in_maps, core_ids=list(range(N_CORES)))

    out = np.empty((B, T1, C), dtype=np.float32)
    for c in range(N_CORES):
        res = last_results.results[c]["out"].reshape(BPC, T1, C)
        for lb, g in enumerate(pairs[c]):
            out[g] = res[lb].astype(np.float32)
    return out



# revision 2
# speedup vs baseline: 1.8715x; 1.8715x over previous
"""ContextMatching kernel for Trainium2, 8-core SPMD — v2.

Math: softmax over j of s[b,i,j] = p1[b,i] + p2[b,j] (masked i<l1, j<l2);
the additive row term p1 cancels inside the softmax, so

    alpha[b,j] = exp(p2[b,j]) / sum_{j'<l2[b]} exp(p2[b,j'])
    out[b,i,:] = (i < l1[b]) ? sum_{j<l2[b]} alpha[b,j] * s2[b,j,:] : 0

s1 never influences the output.  Device computes per batch: p2 = s2 @ w2,
e = exp(p2), masked, ctx = (e @ s2)/sum(e), broadcast ctx to the valid rows.

v2 changes vs v1 (83.7us):
  * Stores: plain HWDGE DMAs predicated with cond= registers (per-core chunk
    counts come in via a tiny int tensor), replacing 16 serialized indirect
    scatters (~45us -> ~6us).  b0 stores issue on sync, b1 on scalar (two
    HWDGE rings).  Invalid rows inside a written chunk are garbage; the host
    copies only rows < l1.
  * Dot products: DVE tensor_tensor (2x mode, ~0.6us) + ACT copy with
    accum_out for the row sum, replacing 1x-mode STT (1.22us serialized).
  * Small statics (w2, masks, selectors, ones) load first in one DMA so
    compute starts as soon as the first s2 chunk lands.
  * Softmax denominator via an extra ones-column matmul into PSUM.
  * Pairing balances (NT, store chunks) via exact DP over matchings.
"""

import os

import numpy as np
import ml_dtypes

BF16 = ml_dtypes.bfloat16

B, T1, T2, C = 16, 1024, 1024, 1024
N_CORES = 8
BPC = B // N_CORES
P = 128
NRO = T1 // P  # output row chunks per batch

_cached = {}
last_results = None


def _build_program(NT):
    import concourse.mybir as mybir
    import concourse.tile as tile
    from concourse import bacc

    f32 = mybir.dt.float32
    bf16 = mybir.dt.bfloat16
    i32 = mybir.dt.int32
    Alu = mybir.AluOpType
    Act = mybir.ActivationFunctionType

    SG = int(os.environ.get("K_SG", "2"))  # store granule, in 128-row chunks
    NG = NRO // SG  # store granules per batch
    P2 = os.environ.get("K_P2", "act")  # p2 via: act (tt+ACT accum) | stt

    # meta layout (bf16 [P, MC]): [0:C]=w2 bcast, [C:C+2NT]=aux interleaved,
    # [C+2NT]=ones col, rows 0-1 of [C+2NT+1 : C+2NT+1+2*P]=selectors
    MC = C + 2 * NT + 1 + 2 * P
    O_AUX = C
    O_ONE = C + 2 * NT
    O_SEL = C + 2 * NT + 1

    nc = bacc.Bacc(None, target_bir_lowering=False, name="context_matching2")

    s2cd = nc.dram_tensor("s2c", [P, NT * C], bf16, kind="ExternalInput")
    metad = nc.dram_tensor("meta", [P, MC], bf16, kind="ExternalInput")
    cndd = nc.dram_tensor("cnd", [1, 2 * NG], i32, kind="ExternalInput")
    outd = [
        nc.dram_tensor(f"out{b}", [T1, C], bf16, kind="ExternalOutput")
        for b in range(BPC)
    ]

    # chunk pairs (pipeline granularity for exp/mask)
    pairs = [list(range(g, min(g + 2, NT))) for g in range(0, NT, 2)]

    with tile.TileContext(nc) as tc:
        with (
            tc.tile_pool(name="statics", bufs=1) as statics,
            tc.tile_pool(name="s2pool", bufs=len(pairs)) as s2pool,
            tc.tile_pool(name="scra", bufs=3) as scra,
            tc.tile_pool(name="scrb", bufs=2) as scrb,
            tc.tile_pool(name="smalls", bufs=1) as smalls,
            tc.tile_pool(name="pctx", bufs=1, space="PSUM") as pctx,
            tc.tile_pool(name="pcb", bufs=1, space="PSUM") as pcb,
        ):
            # ---- loads: small statics first, then s2 chunk groups ----
            meta = statics.tile([P, MC], bf16)
            nc.sync.dma_start(out=meta, in_=metad[:, :])
            cndt = statics.tile([1, 2 * NG], i32)
            nc.scalar.dma_start(out=cndt, in_=cndd[:, :])

            w2b = meta[:, 0:C]
            ones_c = meta[:, O_ONE : O_ONE + 1]
            sel = [meta[0:2, O_SEL + b * P : O_SEL + (b + 1) * P] for b in range(BPC)]

            s2ts = []
            for gi, grp in enumerate(pairs):
                gl = len(grp)
                s2t = s2pool.tile([P, gl * C], bf16, name=f"s2t_{gi}", tag="s2t")
                nc.sync.dma_start(
                    out=s2t, in_=s2cd[:, grp[0] * C : (grp[-1] + 1) * C]
                )
                s2ts.append(s2t)

            # ---- store-predicate registers (loaded early, used at the end) ----
            cregs = []  # [b][w] -> ScalarValue 0/1
            for b in range(BPC):
                eng = nc.sync if b == 0 else nc.scalar
                row = []
                for w in range(NG):
                    r = eng.alloc_register(f"cnd_{b}_{w}")
                    eng.reg_load(r, cndt[0:1, b * NG + w : b * NG + w + 1])
                    row.append(eng.snap(r, min_val=0, max_val=1))
                cregs.append(row)

            # ---- persistent smalls ----
            p2f = smalls.tile([P, NT], f32, name="p2f")
            e = smalls.tile([P, NT], bf16, name="e")
            E2 = smalls.tile([P, NT, 2], bf16, name="E2")
            rinv2 = smalls.tile([2, 1], f32, name="rinv2")
            ctxs = smalls.tile([2, C], bf16, name="ctxs")
            cbs = [smalls.tile([P, C], bf16, name=f"cbs_{b}") for b in range(BPC)]

            ctxp = pctx.tile([2, C], f32, name="ctxp")
            d2p = pctx.tile([2, 1], f32, name="d2p")

            # ---- per-pair pipeline: p2 -> e -> masked e -> ctx matmuls ----
            for gi, grp in enumerate(pairs):
                s2t = s2ts[gi]
                for k, t in enumerate(grp):
                    if P2 == "act":
                        sa = scra.tile([P, C], bf16, name=f"sa_{t}", tag="sa")
                        nc.vector.tensor_tensor(
                            out=sa,
                            in0=s2t[:, k * C : (k + 1) * C],
                            in1=w2b,
                            op=Alu.mult,
                        )
                        sb = scrb.tile([P, C], bf16, name=f"sb_{t}", tag="sb")
                        nc.scalar.activation(
                            out=sb,
                            in_=sa,
                            func=Act.Copy,
                            accum_out=p2f[:, t : t + 1],
                        )
                    else:
                        sa = scra.tile([P, C], bf16, name=f"sa_{t}", tag="sa")
                        nc.vector.scalar_tensor_tensor(
                            out=sa,
                            in0=s2t[:, k * C : (k + 1) * C],
                            scalar=1.0,
                            in1=w2b,
                            op0=Alu.mult,
                            op1=Alu.mult,
                            accum_out=p2f[:, t : t + 1],
                        )
                g0, g1 = grp[0], grp[-1] + 1
                nc.scalar.activation(out=e[:, g0:g1], in_=p2f[:, g0:g1], func=Act.Exp)
                nc.vector.tensor_tensor(
                    out=E2[:, g0:g1, :],
                    in0=e[:, g0:g1].unsqueeze(2).broadcast_to([P, g1 - g0, 2]),
                    in1=meta[:, O_AUX + 2 * g0 : O_AUX + 2 * g1].rearrange(
                        "p (t two) -> p t two", two=2
                    ),
                    op=Alu.mult,
                )
                for k, t in enumerate(grp):
                    st = t == 0
                    sp = t == NT - 1
                    for h in range(2):
                        nc.tensor.matmul(
                            ctxp[:, h * 512 : (h + 1) * 512],
                            lhsT=E2[:, t, :],
                            rhs=s2t[:, k * C + h * 512 : k * C + (h + 1) * 512],
                            start=st,
                            stop=sp,
                        )
                    nc.tensor.matmul(
                        d2p, lhsT=E2[:, t, :], rhs=ones_c, start=st, stop=sp
                    )

            # ---- normalize: rinv = 1/d, ctxs = rinv * ctx (f32->bf16) ----
            nc.vector.reciprocal(rinv2, d2p)
            nc.scalar.activation(
                out=ctxs[:, 0:512], in_=ctxp[:, 0:512], func=Act.Copy,
                scale=rinv2[:, 0:1],
            )
            nc.vector.tensor_scalar_mul(
                out=ctxs[:, 512:1024], in0=ctxp[:, 512:1024], scalar1=rinv2[:, 0:1]
            )

            # ---- broadcast ctx rows to 128 partitions (selector PE) ----
            for b in range(BPC):
                cbp = pcb.tile([P, C], f32, name=f"cbp_{b}", tag=f"cbp{b}")
                for h in range(2):
                    cols = slice(h * 512, (h + 1) * 512)
                    nc.tensor.matmul(
                        cbp[:, cols], lhsT=sel[b], rhs=ctxs[:, cols],
                        start=True, stop=True,
                    )
                nc.scalar.activation(out=cbs[b][:, 0:512], in_=cbp[:, 0:512],
                                     func=Act.Copy)
                nc.vector.tensor_copy(out=cbs[b][:, 512:1024],
                                      in_=cbp[:, 512:1024])

            # ---- predicated contiguous stores (b0 on sync, b1 on scalar) ----
            for b in range(BPC):
                eng = nc.sync if b == 0 else nc.scalar
                src = cbs[b].unsqueeze(1).broadcast_to([P, SG, C])
                ov = outd[b].rearrange("(w g p) c -> w p g c", g=SG, p=P)
                for w in range(NG):
                    eng.dma_start(out=ov[w], in_=src, cond=cregs[b][w])

    nc.finalize()
    return nc


def _plan(l1, l2):
    """Pair batches to minimize (NT, max store chunks) via DP over matchings."""
    kj = (-(-l2 // P)).astype(np.int64)  # ceil(l2/128) >= 1, load chunks
    ki = (-(-l1 // P)).astype(np.int64)  # ceil(l1/128) >= 1, store chunks
    n = len(kj)
    from functools import lru_cache

    @lru_cache(maxsize=None)
    def best(mask):
        if mask == 0:
            return (0, 0, ())
        lo = (mask & -mask).bit_length() - 1
        rest = mask ^ (1 << lo)
        res = None
        mm = rest
        while mm:
            j = (mm & -mm).bit_length() - 1
            mm ^= 1 << j
            sub = best(rest ^ (1 << j))
            cand = (
                max(int(kj[lo] + kj[j]), sub[0]),
                max(int(ki[lo] + ki[j]), sub[1]),
                ((lo, j),) + sub[2],
            )
            if res is None or cand[:2] < res[:2]:
                res = cand
        return res

    nt, _, prs = best((1 << n) - 1)
    # slot0 = larger l1 within each pair
    pairs = [(a, b) if l1[a] >= l1[b] else (b, a) for a, b in prs]
    return pairs, kj, int(nt)


def kernel(s1, l1, s2, l2, w):
    global last_results
    from concourse.bass_utils import run_bass_kernel_spmd

    s2 = np.asarray(s2)
    w = np.asarray(w, dtype=np.float32)
    l1 = np.asarray(l1).astype(np.int64).ravel()
    l2 = np.asarray(l2).astype(np.int64).ravel()
    assert s2.shape == (B, T2, C) and w.shape == (1, 2 * C)

    SG = int(os.environ.get("K_SG", "2"))
    NG = NRO // SG

    pairs, kj, NT = _plan(l1, l2)
    if NT not in _cached:
        _cached[NT] = _build_program(NT)
    nc = _cached[NT]

    MC = C + 2 * NT + 1 + 2 * P
    O_AUX = C
    O_ONE = C + 2 * NT
    O_SEL = C + 2 * NT + 1

    meta = np.zeros((P, MC), dtype=BF16)
    meta[:, 0:C] = np.broadcast_to(w[0, C:].astype(BF16), (P, C))
    meta[:, O_ONE] = 1.0
    for b in range(BPC):
        meta[b, O_SEL + b * P : O_SEL + (b + 1) * P] = 1.0
    iot = np.arange(P)

    in_maps = []
    for c in range(N_CORES):
        s2c = np.zeros((P, NT * C), dtype=BF16)
        m = meta.copy()
        cnd = np.zeros((1, 2 * NG), dtype=np.int32)
        base_t = 0
        for lb, g in enumerate(pairs[c]):
            for k in range(int(kj[g])):
                t = base_t + k
                j0 = k * P
                s2c[:, t * C : (t + 1) * C] = s2[g, j0 : j0 + P, :]
                m[:, O_AUX + 2 * t + lb] = (j0 + iot) < l2[g]
            base_t += int(kj[g])
            cnd[0, lb * NG : (lb + 1) * NG] = (
                np.arange(NG) * SG * P < l1[g]
            ).astype(np.int32)
        in_maps.append({"s2c": s2c, "meta": m, "cnd": cnd})

    last_results = run_bass_kernel_spmd(nc, in_maps, core_ids=list(range(N_CORES)))

    out = np.zeros((B, T1, C), dtype=np.float32)
    for c in range(N_CORES):
        for lb, g in enumerate(pairs[c]):
            nv = int(l1[g])
            res = last_results.results[c][f"out{lb}"]
            out[g, :nv] = res[:nv].astype(np.float32)
    return out
